# revision 32
# baseline (speedup 1.0000x reference)
"""Trainium2 Bass kernel for a DecoderRNN (embedding -> 24-step LSTM -> vocab projection).

Shapes (hardcoded): B=128, T=24, H=E=1024, V=32000, 8 NeuronCores.

End-to-end cost is dominated by host<->device transfer (~60 MB/s axon
tunnel) + host memcpy — device exec is ~1.6 ms — so the layout minimizes
bytes moved (fp16 is used as a pure transfer/wire format; quantizing emb,
W_ih, W_hh, W_out and the logits to fp16 gives rel_l2 ~3e-4 vs the 2e-2
budget, while all matmuls stay float32r):
  - embedding: per-core compaction to the <=384 fp16 rows its 3 steps touch;
  - W_ih/W_hh: each core uploads only its 512-gate-column fp16 shard
    (2.1 MB); ONE merged on-device AllGather replicates the full weights
    (the collective cost model's effective bandwidth ramps up with size, so
    one 16.8 MB AllGather beats two 8.4 MB or several smaller ones);
  - W_out: raw row-shards cast to fp16 on host (no host transpose);
    transposed on the PE via identity matmuls and upcast to fp32r;
  - logits are written/downloaded as fp16 (halves the donated-zeros upload
    and the result download), cast back to fp32 on host.
Compute sharding (device): input projection Xp sharded over steps (3 per
core), assembled with ONE merged fp16 AllGather of all 24 steps; LSTM
recurrence replicated full-batch (PE wall-time is independent of M<=128);
output projection sharded along vocab (4000 columns per core).
"""

import numpy as np

import concourse.bass as bass
import concourse.tile as tile
import concourse.mybir as mybir
from concourse import bacc
from concourse.bass_utils import run_bass_kernel_spmd

B, T = 128, 24
H, E, V = 1024, 1024, 32000
NCORES = 8
TSH = T // NCORES          # 3 Xp step-tiles per core
VSH = V // NCORES          # 4000 vocab columns per core
VPAD = 4096                # W_out shard rows padded to a 128 multiple
VT = 512                   # projection N-tile (8 per core, last 416 valid)
NVT = VPAD // VT
KT = H // 128              # 8 contraction chunks
NT4H = (4 * H) // 512      # 8 gate N-tiles of 512
U = B * TSH                # per-core compacted embedding rows (worst case)

F32 = mybir.dt.float32
F32R = mybir.dt.float32r
F16 = mybir.dt.float16
I32 = mybir.dt.int32

_CACHE = {}


def _gather_weights(nc, tc, w_sh, dram):
    """AllGather the per-core [128, KT, 2, 512] gate-column shard of
    (W_ih^T, W_hh^T) into the full weights on every core (fp16 on the wire,
    upcast to f32r at SBUF load time). One merged collective: the cost
    model's effective bandwidth ramps up with transfer size, so one 16.8 MB
    AllGather beats two 8.4 MB ones.

    Returns (g_w, cc_w): g_w[n, :, k, 0, :] is W_ih^T's (k, n) tile,
    g_w[n, :, k, 1, :] is W_hh^T's."""
    w_bounce = dram.tile([128, KT, 2, 512], F16, tag="wb", name="wb")
    nc.sync.dma_start(w_bounce[:], w_sh[:])
    g_w = dram.tile([NCORES, 128, KT, 2, 512], F16, tag="gw",
                    addr_space="Shared", name="gw")
    cc_w = nc.gpsimd.collective_compute(
        "AllGather", mybir.AluOpType.bypass,
        ins=[w_bounce.opt()], outs=[g_w.opt()],
        replica_groups=[list(range(NCORES))])
    return g_w, cc_w


def _phase_a(nc, tc, tensors):
    """Local Xp step-tiles (j=0..2 -> step c+8j) + one merged AllGather.

    Returns (g, cc): g[r, j, :, :] is step 8j + r's [128, 4096] fp16 Xp."""
    emb_c, caps_l, g_w, cc_w, gbias, onesv, ident, dram = tensors
    with tc.tile_pool(name="a_w", bufs=1) as a_w, \
         tc.tile_pool(name="a_x", bufs=3) as a_x, \
         tc.tile_pool(name="a_xt", bufs=1) as a_xt, \
         tc.tile_pool(name="a_sb", bufs=4) as a_sb, \
         tc.tile_pool(name="a_ps", bufs=4, space="PSUM") as a_ps, \
         tc.tile_pool(name="a_tr", bufs=2, space="PSUM") as a_tr:
        idt = a_w.tile([128, 128], F32)
        nc.sync.dma_start(idt[:], ident[:])
        capst = a_w.tile([128, TSH], I32)
        nc.sync.dma_start(capst[:], caps_l[:])
        # gathers + transposes first so they don't queue behind weight loads
        xt_Ts = []
        gathers = []
        for j in range(TSH):
            x16 = a_x.tile([128, E], F16, tag="x16", name="x16")
            gd = nc.gpsimd.indirect_dma_start(
                out=x16[:], out_offset=None, in_=emb_c[:],
                in_offset=bass.IndirectOffsetOnAxis(ap=capst[:, j:j + 1], axis=0))
            gathers.append(gd)
            x_t = a_x.tile([128, E], F32, tag="x", name="x_t")
            nc.vector.tensor_copy(x_t[:], x16[:])
            xt_T = a_xt.tile([128, KT, 128], F32R, tag=f"xt{j}", name="xt_T")
            for e in range(KT):
                ptr = a_tr.tile([128, 128], F32, tag="tr", name="ptr")
                nc.tensor.transpose(ptr[:], x_t[:, e * 128:(e + 1) * 128], idt[:])
                nc.vector.tensor_copy(xt_T[:, e, :], ptr[:])
            xt_Ts.append(xt_T)
        gbr = a_w.tile([1, 4 * H], F32R)
        nc.sync.dma_start(gbr[:], gbias[None, :])
        ones1 = a_w.tile([1, 128], F32R)
        nc.sync.dma_start(ones1[:], onesv[None, :])
        # full W_ih^T tiles from the weight AllGather (fp16 -> f32r upcast),
        # in matmul use order
        wih = {}
        for n in range(NT4H):
            for k in range(KT):
                wt16 = a_x.tile([128, 512], F16, tag="w16", name="w16")
                dma = nc.sync.dma_start(wt16[:], g_w[n, :, k, 0, :])
                tile.add_dep_helper(dma.ins, cc_w.ins, sync=True,
                                    reason="W_ih read after weight AllGather")
                tile.add_dep_helper(dma.ins, gathers[-1].ins, sync=True,
                                    reason="weight stream yields to critical loads")
                wt = a_w.tile([128, 512], F32R, tag=f"wi{n}_{k}", name="wi")
                nc.vector.tensor_copy(wt[:], wt16[:])
                wih[(n, k)] = wt

        bounce_in = dram.tile([TSH, 128, 4 * H], F16, tag="agin")
        for j in range(TSH):
            xt_T = xt_Ts[j]
            for n in range(NT4H):
                ns = slice(n * 512, (n + 1) * 512)
                ps = a_ps.tile([128, 512], F32, tag="ps", name="ps")
                nc.tensor.matmul(ps[:], ones1[:, :], gbr[:, ns],
                                 start=True, stop=False)
                for k in range(KT):
                    nc.tensor.matmul(ps[:], xt_T[:, k, :], wih[(n, k)][:],
                                     start=False, stop=(k == KT - 1))
                xp_sb = a_sb.tile([128, 512], F16, tag="xp", name="xp_sb")
                nc.vector.tensor_copy(xp_sb[:], ps[:])
                nc.sync.dma_start(bounce_in[j, :, ns], xp_sb[:])
        # one merged AllGather for all 3 step-tiles (size rides the
        # bandwidth ramp; 1 fixed overhead instead of 3)
        g = dram.tile([NCORES, TSH, 128, 4 * H], F16, tag="agout",
                      addr_space="Shared", name="agout")
        cc = nc.gpsimd.collective_compute(
            "AllGather", mybir.AluOpType.bypass,
            ins=[bounce_in.opt()], outs=[g.opt()],
            replica_groups=[list(range(NCORES))])
    return g, cc


def _phase_b(nc, tc, tensors):
    """24 serial LSTM steps; h^T history to DRAM."""
    g_w, cc_w, feats, ident, xp_g, hT_dram = tensors
    g_xp, cc_xp = xp_g
    with tc.tile_pool(name="b_w", bufs=1) as b_w, \
         tc.tile_pool(name="b_xp", bufs=17) as b_xp, \
         tc.tile_pool(name="b_act", bufs=1) as b_act, \
         tc.tile_pool(name="b_tmp", bufs=2) as b_tmp, \
         tc.tile_pool(name="b_ps", bufs=6, space="PSUM") as b_ps, \
         tc.tile_pool(name="b_tr", bufs=2, space="PSUM") as b_tr:
        idt = b_w.tile([128, 128], F32)
        nc.sync.dma_start(idt[:], ident[:])
        c_st = b_w.tile([128, H], F32)
        nc.sync.dma_start(c_st[:], feats[:])
        tnh = b_w.tile([128, H], F32)
        h_t = b_w.tile([128, H], F32)
        # hT double-buffered across steps: gate matmuls of step t read h_{t-1}^T
        # from one buffer while the new h_t^T transposes land in the other.
        hT_a = b_w.tile([128, KT, 128], F32R, tag="hT0")
        hT_b = b_w.tile([128, KT, 128], F32R, tag="hT1")
        hT_bufs = [hT_a, hT_b]
        # h0^T = feats^T via PE transposes (no host-side transpose upload)
        for e in range(KT):
            ptr = b_tr.tile([128, 128], F32, tag="tr", name="ptr")
            nc.tensor.transpose(ptr[:], c_st[:, e * 128:(e + 1) * 128], idt[:])
            nc.vector.tensor_copy(hT_bufs[0][:, e, :], ptr[:])

        # gate cols: i [0,1024) f [1024,2048) g [2048,3072) o [3072,4096)
        ACT_FN = {0: "Sigmoid", 1: "Sigmoid", 2: "Sigmoid", 3: "Sigmoid",
                  4: "Tanh", 5: "Tanh", 6: "Sigmoid", 7: "Sigmoid"}
        N_ORDER = (0, 4, 2, 1, 5, 3, 6, 7)

        def xp_load(t, n, ret_dma=False):
            ns = slice(n * 512, (n + 1) * 512)
            xp_n = b_xp.tile([128, 512], F16, tag="xpn", name="xp_n")
            # ACT's HWDGE ring, so these don't queue behind the weight
            # stream on the sync ring.
            dma = nc.scalar.dma_start(xp_n[:], g_xp[t % 8, t // 8, :, ns])
            # Tile does not order reads of the AllGather output after the
            # collective on its own; pin the edge explicitly.
            tile.add_dep_helper(dma.ins, cc_xp.ins, sync=True,
                                reason="xp read after AllGather")
            return (xp_n, dma) if ret_dma else xp_n

        # prefetch the first two steps' xp slices ahead of the weight stream
        xp_pre = {}
        xp_pre_dmas = []
        for t_pre in range(2):
            for n in N_ORDER:
                xp_n, dma = xp_load(t_pre, n, ret_dma=True)
                xp_pre[(t_pre, n)] = xp_n
                xp_pre_dmas.append(dma)
        # full W_hh^T tiles from the weight AllGather (fp16 -> f32r upcast),
        # in matmul use order
        whh = {}
        for n in N_ORDER:
            for k in range(KT):
                wt16 = b_tmp.tile([128, 512], F16, tag="w16", name="w16")
                dma = nc.sync.dma_start(wt16[:], g_w[n, :, k, 1, :])
                tile.add_dep_helper(dma.ins, cc_w.ins, sync=True,
                                    reason="W_hh read after weight AllGather")
                tile.add_dep_helper(dma.ins, xp_pre_dmas[-1].ins, sync=True,
                                    reason="weight stream yields to xp prefetch")
                wt = b_w.tile([128, 512], F32R, tag=f"wh{n}_{k}", name="wh")
                nc.vector.tensor_copy(wt[:], wt16[:])
                whh[(n, k)] = wt

        def act_gate(n, a_t, ps):
            ns = slice(n * 512, (n + 1) * 512)
            nc.scalar.activation(a_t[:, ns], ps[:],
                                 getattr(mybir.ActivationFunctionType, ACT_FN[n]))

        def gate_mms(t, n, a_t, hT_src):
            xp_n = xp_pre.pop((t, n), None)
            if xp_n is None:
                xp_n = xp_load(t, n)
            ps = b_ps.tile([128, 512], F32, tag="ps", name="ps")
            for k in range(KT):
                nc.tensor.matmul(ps[:], hT_src[:, k, :], whh[(n, k)][:],
                                 start=(k == 0), stop=(k == KT - 1))
            nc.vector.tensor_add(ps[:], ps[:], xp_n[:])
            act_gate(n, a_t, ps)

        def cell_half(half, a_t):
            hs = slice(half * 512, half * 512 + 512)
            ig = b_tmp.tile([128, 512], F32, tag="ig", name="ig")
            nc.vector.tensor_mul(ig[:], a_t[:, half * 512:half * 512 + 512],
                                 a_t[:, 2 * H + half * 512:2 * H + half * 512 + 512])
            fc = b_tmp.tile([128, 512], F32, tag="fc", name="fc")
            nc.vector.tensor_mul(fc[:], a_t[:, H + half * 512:H + half * 512 + 512],
                                 c_st[:, hs])
            nc.vector.tensor_add(c_st[:, hs], ig[:], fc[:])
            nc.scalar.activation(tnh[:, hs], c_st[:, hs],
                                 mybir.ActivationFunctionType.Tanh)

        def h_half(half, a_t, hT_dst):
            # quarter granularity: the first transposes start while the DVE
            # is still producing the second quarter of h
            for q in range(2):
                qs = half * 512 + q * 256
                nc.vector.tensor_mul(h_t[:, qs:qs + 256],
                                     a_t[:, 3 * H + qs:3 * H + qs + 256],
                                     tnh[:, qs:qs + 256])
                for e in range(half * 4 + q * 2, half * 4 + q * 2 + 2):
                    ptr = b_tr.tile([128, 128], F32, tag="tr", name="ptr")
                    nc.tensor.transpose(ptr[:], h_t[:, e * 128:(e + 1) * 128], idt[:])
                    nc.vector.tensor_copy(hT_dst[:, e, :], ptr[:])

        for t in range(T):
            hT_src = hT_bufs[t % 2]
            hT_dst = hT_bufs[(t + 1) % 2]
            a_t = b_act.tile([128, 4 * H], F32, tag="a", name="a_t")
            for n in (0, 4, 2):
                gate_mms(t, n, a_t, hT_src)
            cell_half(0, a_t)
            for n in (1, 5, 3):
                gate_mms(t, n, a_t, hT_src)
            cell_half(1, a_t)
            gate_mms(t, 6, a_t, hT_src)
            h_half(0, a_t, hT_dst)
            gate_mms(t, 7, a_t, hT_src)
            h_half(1, a_t, hT_dst)
            nc.scalar.dma_start(hT_dram[t, :, :], hT_dst.rearrange("p k b -> p (k b)"))


def _phase_c(nc, tc, tensors):
    """logits = h @ W_out^T + b_out for this core's vocab shard.

    W_out arrives as raw fp16 rows [VPAD, H]; each 512-row vocab tile is
    PE-transposed (identity matmul) and upcast to f32r tiles, hidden behind
    the 40 us of matmuls each vocab tile feeds. All 24 h^T step tiles are
    loaded once and stay resident. Output is written as fp16."""
    w_out16, b_out, ident, hT_dram, out_c = tensors
    with tc.tile_pool(name="c_w", bufs=1) as c_w, \
         tc.tile_pool(name="c_wv", bufs=8) as c_wv, \
         tc.tile_pool(name="c_wo", bufs=2) as c_wo, \
         tc.tile_pool(name="c_h", bufs=1) as c_h, \
         tc.tile_pool(name="c_ob", bufs=6) as c_ob, \
         tc.tile_pool(name="c_ps", bufs=6, space="PSUM") as c_ps, \
         tc.tile_pool(name="c_tr", bufs=2, space="PSUM") as c_tr:
        idt = c_w.tile([128, 128], F32)
        nc.sync.dma_start(idt[:], ident[:])
        bo = c_w.tile([128, VPAD], F32)
        nc.gpsimd.dma_start(out=bo[:], in_=b_out[None, :].to_broadcast([128, VPAD]))
        hTts = []
        for t in range(T):
            hTt = c_h.tile([128, KT, 128], F32R, tag=f"ht{t}", name="hTt")
            nc.scalar.dma_start(
                hTt[:], hT_dram[t, :, :].rearrange("p (k b) -> p k b", k=KT))
            hTts.append(hTt)

        for n in range(NVT):
            vs0 = n * VT
            # load 4x[128, H] fp16 row blocks, upcast, PE-transpose into
            # 8 f32r [128(k), VT] weight tiles
            wv32s = []
            for i in range(4):
                wv = c_wv.tile([128, H], F16, tag="wv", name="wv")
                nc.sync.dma_start(wv[:], w_out16[vs0 + i * 128:vs0 + (i + 1) * 128, :])
                wv32 = c_wv.tile([128, H], F32, tag="wv32", name="wv32")
                nc.vector.tensor_copy(wv32[:], wv[:])
                wv32s.append(wv32)
            wos = []
            for k in range(KT):
                wo = c_wo.tile([128, VT], F32R, tag=f"wo{k}", name="wo")
                for i in range(4):
                    ptr = c_tr.tile([128, 128], F32, tag="tr", name="ptr")
                    nc.tensor.transpose(ptr[:], wv32s[i][:, k * 128:(k + 1) * 128],
                                        idt[:])
                    nc.vector.tensor_copy(wo[:, i * 128:(i + 1) * 128], ptr[:])
                wos.append(wo)
            w_valid = min(VT, VSH - vs0)
            for t in range(T):
                ps = c_ps.tile([128, VT], F32, tag="ps", name="ps")
                for k in range(KT):
                    nc.tensor.matmul(ps[:], hTts[t][:, k, :], wos[k][:, :],
                                     start=(k == 0), stop=(k == KT - 1))
                nc.vector.tensor_add(ps[:], ps[:], bo[:, vs0:vs0 + VT])
                ob = c_ob.tile([128, VT], F16, tag="ob", name="ob")
                nc.scalar.activation(ob[:], ps[:],
                                     mybir.ActivationFunctionType.Copy)
                nc.sync.dma_start(out_c[:, t, vs0:vs0 + w_valid], ob[:, :w_valid])


def _build(variant: str = "full"):
    """variant: "full" or "null" (I/O-only, for wall-clock delta timing)."""
    nc = bacc.Bacc("TRN2", target_bir_lowering=False, debug=False)

    emb_c = nc.dram_tensor("emb_c", [U, E], F16, kind="ExternalInput")
    caps_l = nc.dram_tensor("caps_l", [B, TSH], I32, kind="ExternalInput")
    w_sh = nc.dram_tensor("w_sh", [128, KT, 2, 512], F16, kind="ExternalInput")
    gbias = nc.dram_tensor("gbias", [4 * H], F32R, kind="ExternalInput")
    onesv = nc.dram_tensor("onesv", [128], F32R, kind="ExternalInput")
    w_out16 = nc.dram_tensor("w_out16", [VPAD, H], F16, kind="ExternalInput")
    b_out = nc.dram_tensor("b_out", [VPAD], F32, kind="ExternalInput")
    feats = nc.dram_tensor("feats", [B, H], F32, kind="ExternalInput")
    ident = nc.dram_tensor("ident", [128, 128], F32, kind="ExternalInput")
    out_c = nc.dram_tensor("out_c", [B, T, VSH], F16, kind="ExternalOutput")

    hT_dram = nc.dram_tensor("hT_dram", [T, 128, KT * 128], F32R)

    if variant == "null":
        with tile.TileContext(nc) as tc:
            with tc.tile_pool(name="p", bufs=2) as pool:
                t0 = pool.tile([128, VT], F16)
                nc.sync.dma_start(t0[:], w_out16[0:128, 0:VT])
                for t in range(T):
                    nc.sync.dma_start(out_c[:, t, 0:VT], t0[:])
        nc.compile()
        return nc

    with tile.TileContext(nc) as tc:
        with tc.tile_pool(name="dram", bufs=1, space="DRAM") as dram:
            g_w, cc_w = _gather_weights(nc, tc, w_sh, dram)
            xp_g = _phase_a(nc, tc, (emb_c, caps_l, g_w, cc_w, gbias, onesv,
                                     ident, dram))
            _phase_b(nc, tc, (g_w, cc_w, feats, ident, xp_g, hT_dram))
            _phase_c(nc, tc, (w_out16, b_out, ident, hT_dram, out_c))

    nc.compile()
    return nc


def _prep_inputs(features, captions, emb, W_ih, W_hh, b_ih, b_hh, W_out, b_out):
    """Host-side sharding. Deliberately avoids any large transpose/copy:
    every per-core array is either a contiguous slice, a small strided
    copy (2.1 MB fp16 weight shards), or a cast (fp16 shards). Per-core
    work runs on a thread pool."""
    from concurrent.futures import ThreadPoolExecutor

    features = np.asarray(features, np.float32)
    captions = np.asarray(captions)
    emb = np.asarray(emb, np.float32)
    W_ih = np.asarray(W_ih, np.float32)
    W_hh = np.asarray(W_hh, np.float32)
    gbias = (np.asarray(b_ih, np.float32) + np.asarray(b_hh, np.float32))
    W_out = np.asarray(W_out, np.float32)
    b_out = np.asarray(b_out, np.float32)

    common = {
        "gbias": gbias, "onesv": np.ones(128, np.float32),
        "feats": features, "ident": np.eye(128, dtype=np.float32),
    }

    def core_prep(c):
        # embedding rows for this core's 3 steps only, remapped indices
        steps = [c + 8 * j for j in range(TSH)]
        capsc = captions[:, steps]
        uniq, inv = np.unique(capsc.reshape(-1), return_inverse=True)
        emb_cc = np.zeros((U, E), np.float16)
        emb_cc[:uniq.shape[0]] = emb[uniq]
        # this core's 512 gate columns of W_ih^T / W_hh^T in PE layout:
        # w_sh[p, k, 0, m] = W_ih[c*512 + m, k*128 + p]
        ms = slice(c * 512, (c + 1) * 512)
        w_sh = np.empty((128, KT, 2, 512), np.float16)
        w_sh[:, :, 0, :] = W_ih[ms].reshape(512, KT, 128).transpose(2, 1, 0)
        w_sh[:, :, 1, :] = W_hh[ms].reshape(512, KT, 128).transpose(2, 1, 0)
        # raw fp16 W_out rows (device transposes); zero-padded to VPAD
        w_out16 = np.zeros((VPAD, H), np.float16)
        w_out16[:VSH] = W_out[c * VSH:(c + 1) * VSH]
        b_out_p = np.zeros(VPAD, np.float32)
        b_out_p[:VSH] = b_out[c * VSH:(c + 1) * VSH]
        return {
            "emb_c": emb_cc,
            "caps_l": np.ascontiguousarray(inv.reshape(B, TSH).astype(np.int32)),
            "w_sh": w_sh,
            "w_out16": w_out16,
            "b_out": b_out_p,
        }

    with ThreadPoolExecutor(NCORES) as ex:
        per_core = list(ex.map(core_prep, range(NCORES)))
    return common, per_core


def kernel(**inputs) -> np.ndarray:
    common, per_core = _prep_inputs(**inputs)

    if "full" not in _CACHE:
        _CACHE["full"] = _build("full")
    nc = _CACHE["full"]

    in_maps = [dict(common, **pc) for pc in per_core]
    res = run_bass_kernel_spmd(nc, in_maps, core_ids=list(range(NCORES)))

    from concurrent.futures import ThreadPoolExecutor

    out = np.empty((B, T + 1, V), np.float32)
    out[:, 0, :] = 0.0
    out[:, 0, 1] = 1.0

    def core_out(c):
        out[:, 1:, c * VSH:(c + 1) * VSH] = res.results[c]["out_c"]

    with ThreadPoolExecutor(NCORES) as ex:
        list(ex.map(core_out, range(NCORES)))
    return out
